# revision 12
# baseline (speedup 1.0000x reference)
"""MergeDecoder GNN edge-scorer for Trainium2 (8 NeuronCores).

Reference computation (per edge e):
    x  = [z[src[e]] ; z[dst[e]]]            # [512]
    h  = relu(w1 @ x + b1)                  # [256]
    out[e] = w2 @ h + b2                    # [1]

Strategy (matches sharding hint: data-parallel over E, replicate z/weights):
  - Host: cast z to bf16, shard edges 8-way, bucket-sort each shard's edges
    by (src>=32768, dst>=32768) so relative row indices fit the gather's
    int16 index format; build a unified static chunk schedule.
  - Device (per core): transposed dma_gather pulls z rows FEATURE-MAJOR into
    SBUF ([128 feat, 2, chunk_edges]); per 512-edge group, 8 accumulating
    matmuls compute h^T in PSUM; ScalarE fuses +b1, ReLU, bf16 cast on the
    PSUM->SBUF move; two M=1 matmuls against w2 produce scores [1,512] in
    PSUM; ScalarE adds b2 on the copy out. One DMA writes all scores back.
  - Host: de-interleave group-major scores, undo the bucket permutation.
"""

import os
import sys

for _p in ("/opt/trn_rl_repo",):
    if _p not in sys.path and os.path.isdir(_p):
        sys.path.insert(0, _p)

import numpy as np
import ml_dtypes

N, D, E = 50000, 256, 320000
NCORES = 8
EPC = E // NCORES          # 40000 edges per core
H = 32768                  # int16-safe split of node rows
NPAD = 50048               # z rows padded to a multiple of 128
CHUNK = 4096               # max edges per dma_gather call
GROUP = 512                # edges per matmul group (PSUM bank width in f32)

BF16 = ml_dtypes.bfloat16

_cache = {}


def _build(schedule, L, Sw):
    """Trace the per-core Bass program for a given static chunk schedule."""
    from contextlib import ExitStack

    import concourse.bass as bass
    import concourse.mybir as mybir
    import concourse.tile as tile
    from concourse import bacc

    FT = mybir.ActivationFunctionType
    dt = mybir.dt

    nc = bacc.Bacc("TRN2", target_bir_lowering=False, debug=False,
                   enable_asserts=False)

    z_d = nc.dram_tensor("z", [NPAD, D], dt.bfloat16, kind="ExternalInput")
    isrc_d = nc.dram_tensor("isrc", [128, L // 16], dt.int16, kind="ExternalInput")
    idst_d = nc.dram_tensor("idst", [128, L // 16], dt.int16, kind="ExternalInput")
    w1t_d = nc.dram_tensor("w1t", [128, 4 * D], dt.bfloat16, kind="ExternalInput")
    b1t_d = nc.dram_tensor("b1t", [128, 2], dt.float32, kind="ExternalInput")
    w2t_d = nc.dram_tensor("w2t", [128, 2], dt.bfloat16, kind="ExternalInput")
    b2t_d = nc.dram_tensor("b2t", [1, 1], dt.float32, kind="ExternalInput")
    out_d = nc.dram_tensor("scores", [1, L], dt.float32, kind="ExternalOutput")

    with tile.TileContext(nc) as tc, ExitStack() as ctx:
        consts = ctx.enter_context(tc.tile_pool(name="consts", bufs=1))
        gpool_s = ctx.enter_context(tc.tile_pool(name="gath_s", bufs=8))
        gpool_d = ctx.enter_context(tc.tile_pool(name="gath_d", bufs=8))
        htpool = ctx.enter_context(tc.tile_pool(name="ht", bufs=4))
        hppool = ctx.enter_context(tc.tile_pool(name="hpsum", bufs=4, space="PSUM"))
        sppool = ctx.enter_context(tc.tile_pool(name="spsum", bufs=2, space="PSUM"))
        scpool = ctx.enter_context(tc.tile_pool(name="scores", bufs=2))

        w1t = consts.tile([128, 4 * D], dt.bfloat16)
        nc.sync.dma_start(w1t[:], w1t_d.ap())
        b1t = consts.tile([128, 2], dt.float32)
        nc.sync.dma_start(b1t[:], b1t_d.ap())
        w2t = consts.tile([128, 2], dt.bfloat16)
        nc.sync.dma_start(w2t[:], w2t_d.ap())
        b2t = consts.tile([1, 1], dt.float32)
        nc.sync.dma_start(b2t[:], b2t_d.ap())
        isrc = consts.tile([128, L // 16], dt.int16)
        nc.sync.dma_start(isrc[:], isrc_d.ap())
        idst = consts.tile([128, L // 16], dt.int16)
        nc.sync.dma_start(idst[:], idst_d.ap())

        z_lo = z_d.ap()[0:H, :]
        z_hi = z_d.ap()[H:NPAD, :]

        j0 = 0
        for (b, clen) in schedule:
            src_base = z_hi if (b >> 1) else z_lo
            dst_base = z_hi if (b & 1) else z_lo

            sc = scpool.tile([1, CHUNK], dt.float32, tag="sc",
                             name=f"sc_{j0}")
            for gg in range(clen // GROUP):
                j = j0 + gg * GROUP
                # transpose-gather rx needs 2*n/16+2 of the 128
                # descriptor-ring slots per DMA engine -> n <= 896/call;
                # use one GROUP-sized call per matmul group
                xs = gpool_s.tile([128, 2 * GROUP], dt.bfloat16, tag="xs",
                                  name=f"xs_{j}")
                xd = gpool_d.tile([128, 2 * GROUP], dt.bfloat16, tag="xd",
                                  name=f"xd_{j}")
                for (x_t, base, idx) in ((xs, src_base, isrc),
                                         (xd, dst_base, idst)):
                    nc.gpsimd.dma_gather(
                        x_t[:].rearrange("p (h c) -> p h c", h=2),
                        base,
                        idx[:, j // 16:(j + GROUP) // 16],
                        GROUP,
                        GROUP,
                        D,
                        elem_step=D,
                        transpose=True,
                    )
                ht = [None, None]
                for m in range(2):
                    hp = hppool.tile([128, GROUP], dt.float32)
                    for t in range(4):
                        x_t = xs if t < 2 else xd
                        h = t & 1
                        nc.tensor.matmul(
                            out=hp[:],
                            lhsT=w1t[:, t * D + 128 * m: t * D + 128 * m + 128],
                            rhs=x_t[:, h * GROUP:(h + 1) * GROUP],
                            start=(t == 0),
                            stop=(t == 3),
                        )
                    ht[m] = htpool.tile([128, GROUP], dt.bfloat16, tag="ht",
                                        name=f"ht_{j}_{m}")
                    # bias+relu+cast fused on the PSUM->SBUF move; split the
                    # two halves across ScalarE and VectorE
                    if m == 0:
                        nc.scalar.activation(ht[m][:], hp[:], FT.Relu,
                                             bias=b1t[:, m:m + 1])
                    else:
                        nc.vector.tensor_scalar(
                            ht[m][:], hp[:], b1t[:, m:m + 1], 0.0,
                            mybir.AluOpType.add, mybir.AluOpType.max)
                sp = sppool.tile([1, GROUP], dt.float32)
                nc.tensor.matmul(out=sp[:], lhsT=w2t[:, 0:1], rhs=ht[0][:],
                                 start=True, stop=False)
                nc.tensor.matmul(out=sp[:], lhsT=w2t[:, 1:2], rhs=ht[1][:],
                                 start=False, stop=True)
                sc_slice = sc[0:1, gg * GROUP:(gg + 1) * GROUP]
                if gg % 2 == 0:
                    nc.scalar.activation(sc_slice, sp[:], FT.Identity,
                                         bias=b2t[:, :])
                else:
                    nc.vector.tensor_scalar_add(sc_slice, sp[:], b2t[:, :])
            nc.sync.dma_start(out_d.ap()[0:1, j0:j0 + clen], sc[0:1, 0:clen])
            j0 += clen

    nc.compile()
    return nc


def _prep(inputs):
    z = np.asarray(inputs["z"], np.float32)
    ei = np.asarray(inputs["edge_index"]).astype(np.int64)
    w1 = np.asarray(inputs["w1"], np.float32)
    b1 = np.asarray(inputs["b1"], np.float32)
    w2 = np.asarray(inputs["w2"], np.float32)
    b2 = np.asarray(inputs["b2"], np.float32)

    zb = np.zeros((NPAD, D), BF16)
    zb[:N] = z.astype(BF16)

    w1t = np.ascontiguousarray(
        w1.T.reshape(4, 128, D).transpose(1, 0, 2).reshape(128, 4 * D)
    ).astype(BF16)
    b1t = np.ascontiguousarray(b1.reshape(2, 128).T)
    w2t = np.ascontiguousarray(w2.reshape(2, 128).T).astype(BF16)
    b2t = b2.reshape(1, 1).astype(np.float32)

    src = ei[0].reshape(NCORES, EPC)
    dst = ei[1].reshape(NCORES, EPC)
    bucket = (2 * (src >= H) + (dst >= H)).astype(np.int8)

    counts = np.stack([(bucket == b).sum(axis=1) for b in range(4)], axis=1)
    caps = [int(-(-counts[:, b].max() // GROUP)) * GROUP for b in range(4)]
    schedule = []
    for b in range(4):
        rem = caps[b]
        while rem > 0:
            c = min(CHUNK, rem)
            schedule.append((b, c))
            rem -= c
    L = sum(caps)
    G = L // GROUP
    Gb = G
    Sw = L

    isrc = np.zeros((NCORES, L), np.int16)
    idst = np.zeros((NCORES, L), np.int16)
    eids = np.full((NCORES, L), -1, np.int64)
    for c in range(NCORES):
        pos = 0
        for b in range(4):
            sel = np.nonzero(bucket[c] == b)[0]
            n = len(sel)
            isrc[c, pos:pos + n] = (src[c, sel] - H * (b >> 1)).astype(np.int16)
            idst[c, pos:pos + n] = (dst[c, sel] - H * (b & 1)).astype(np.int16)
            eids[c, pos:pos + n] = sel
            pos += caps[b]

    # wrapped layout [16, L/16] (idx j -> [j%16, j//16]), replicated to all
    # 128 partitions (8 gpsimd Q7 cores x 16 partitions each)
    isrc_w = np.ascontiguousarray(np.tile(
        isrc.reshape(NCORES, L // 16, 16).transpose(0, 2, 1), (1, 8, 1)))
    idst_w = np.ascontiguousarray(np.tile(
        idst.reshape(NCORES, L // 16, 16).transpose(0, 2, 1), (1, 8, 1)))

    in_maps = []
    for c in range(NCORES):
        in_maps.append({
            "z": zb, "isrc": isrc_w[c], "idst": idst_w[c],
            "w1t": w1t, "b1t": b1t, "w2t": w2t, "b2t": b2t,
        })
    meta = dict(schedule=tuple(schedule), L=L, Sw=Sw, Gb=Gb, eids=eids)
    return in_maps, meta


def _post(results, meta):
    L, Gb, eids = meta["L"], meta["Gb"], meta["eids"]
    out = np.empty((E, 1), np.float32)
    for c in range(NCORES):
        sj = results[c]["scores"].reshape(-1)[:L]
        valid = eids[c] >= 0
        res = np.empty(EPC, np.float32)
        res[eids[c, valid]] = sj[valid]
        out[c * EPC:(c + 1) * EPC, 0] = res
    return out


def kernel(**inputs) -> np.ndarray:
    from concourse import bass_utils

    in_maps, meta = _prep(inputs)
    key = (meta["schedule"], meta["L"], meta["Sw"])
    if key not in _cache:
        _cache[key] = _build(list(meta["schedule"]), meta["L"], meta["Sw"])
    nc = _cache[key]
    res = bass_utils.run_bass_kernel_spmd(nc, in_maps, core_ids=list(range(NCORES)))
    return _post(res.results, meta)


# revision 13
# speedup vs baseline: 1.9757x; 1.9757x over previous
"""MergeDecoder GNN edge-scorer for Trainium2 (8 NeuronCores).

Reference computation (per edge e):
    x  = [z[src[e]] ; z[dst[e]]]            # [512]
    h  = relu(w1 @ x + b1)                  # [256]
    out[e] = w2 @ h + b2                    # [1]

Strategy (matches sharding hint: data-parallel over E, replicate z/weights):
  - Host: cast z to bf16, shard edges 8-way, bucket-sort each shard's edges
    by (src>=32768, dst>=32768) so relative row indices fit the gather's
    int16 index format; build a unified static chunk schedule.
  - Device (per core): transposed dma_gather pulls z rows FEATURE-MAJOR into
    SBUF ([128 feat, 2, chunk_edges]); per 512-edge group, 8 accumulating
    matmuls compute h^T in PSUM; ScalarE fuses +b1, ReLU, bf16 cast on the
    PSUM->SBUF move; two M=1 matmuls against w2 produce scores [1,512] in
    PSUM; ScalarE adds b2 on the copy out. One DMA writes all scores back.
  - Host: de-interleave group-major scores, undo the bucket permutation.
"""

import os
import sys

for _p in ("/opt/trn_rl_repo",):
    if _p not in sys.path and os.path.isdir(_p):
        sys.path.insert(0, _p)

import numpy as np
import ml_dtypes

N, D, E = 50000, 256, 320000
NCORES = 8
EPC = E // NCORES          # 40000 edges per core
H = 32768                  # int16-safe split of node rows
NPAD = 50048               # z rows padded to a multiple of 128
CHUNK = 4096               # max edges per dma_gather call
GROUP = 512                # edges per matmul group (PSUM bank width in f32)

BF16 = ml_dtypes.bfloat16

_cache = {}


def _build(schedule, L, Sw):
    """Trace the per-core Bass program for a given static chunk schedule."""
    from contextlib import ExitStack

    import concourse.bass as bass
    import concourse.mybir as mybir
    import concourse.tile as tile
    from concourse import bacc

    FT = mybir.ActivationFunctionType
    dt = mybir.dt

    nc = bacc.Bacc("TRN2", target_bir_lowering=False, debug=False,
                   enable_asserts=False)

    z_d = nc.dram_tensor("z", [NPAD, D], dt.bfloat16, kind="ExternalInput")
    isrc_d = nc.dram_tensor("isrc", [128, L // 16], dt.int16, kind="ExternalInput")
    idst_d = nc.dram_tensor("idst", [128, L // 16], dt.int16, kind="ExternalInput")
    w1t_d = nc.dram_tensor("w1t", [128, 4 * D], dt.bfloat16, kind="ExternalInput")
    b1t_d = nc.dram_tensor("b1t", [128, 2], dt.float32, kind="ExternalInput")
    w2t_d = nc.dram_tensor("w2t", [128, 2], dt.bfloat16, kind="ExternalInput")
    b2t_d = nc.dram_tensor("b2t", [1, 1], dt.float32, kind="ExternalInput")
    out_d = nc.dram_tensor("scores", [1, L], dt.float32, kind="ExternalOutput")

    with tile.TileContext(nc) as tc, ExitStack() as ctx:
        consts = ctx.enter_context(tc.tile_pool(name="consts", bufs=1))
        gpool_s = ctx.enter_context(tc.tile_pool(name="gath_s", bufs=8))
        gpool_d = ctx.enter_context(tc.tile_pool(name="gath_d", bufs=8))
        htpool = ctx.enter_context(tc.tile_pool(name="ht", bufs=4))
        hppool = ctx.enter_context(tc.tile_pool(name="hpsum", bufs=4, space="PSUM"))
        sppool = ctx.enter_context(tc.tile_pool(name="spsum", bufs=2, space="PSUM"))
        scpool = ctx.enter_context(tc.tile_pool(name="scores", bufs=2))

        w1t = consts.tile([128, 4 * D], dt.bfloat16)
        nc.sync.dma_start(w1t[:], w1t_d.ap())
        b1t = consts.tile([128, 2], dt.float32)
        nc.sync.dma_start(b1t[:], b1t_d.ap())
        w2t = consts.tile([128, 2], dt.bfloat16)
        nc.sync.dma_start(w2t[:], w2t_d.ap())
        b2t = consts.tile([1, 1], dt.float32)
        nc.sync.dma_start(b2t[:], b2t_d.ap())
        isrc = consts.tile([128, L // 16], dt.int16)
        nc.sync.dma_start(isrc[:], isrc_d.ap())
        idst = consts.tile([128, L // 16], dt.int16)
        nc.sync.dma_start(idst[:], idst_d.ap())

        z_lo = z_d.ap()[0:H, :]
        z_hi = z_d.ap()[H:NPAD, :]

        j0 = 0
        for (b, clen) in schedule:
            src_base = z_hi if (b >> 1) else z_lo
            dst_base = z_hi if (b & 1) else z_lo
            jw0 = j0 % L  # wraps only for repeated benchmark schedules

            sc = scpool.tile([1, CHUNK], dt.float32, tag="sc",
                             name=f"sc_{j0}")
            for gg in range(clen // GROUP):
                j = jw0 + gg * GROUP
                # transpose-gather rx needs 2*n/16+2 of the 128
                # descriptor-ring slots per DMA engine -> n <= 896/call;
                # use one GROUP-sized call per matmul group
                xs = gpool_s.tile([128, 2 * GROUP], dt.bfloat16, tag="xs",
                                  name=f"xs_{j}")
                xd = gpool_d.tile([128, 2 * GROUP], dt.bfloat16, tag="xd",
                                  name=f"xd_{j}")
                for (x_t, base, idx) in ((xs, src_base, isrc),
                                         (xd, dst_base, idst)):
                    nc.gpsimd.dma_gather(
                        x_t[:].rearrange("p (h c) -> p h c", h=2),
                        base,
                        idx[:, j // 16:(j + GROUP) // 16],
                        GROUP,
                        GROUP,
                        D,
                        elem_step=D,
                        transpose=True,
                    )
                ht = [None, None]
                for m in range(2):
                    hp = hppool.tile([128, GROUP], dt.float32)
                    for t in range(4):
                        x_t = xs if t < 2 else xd
                        h = t & 1
                        nc.tensor.matmul(
                            out=hp[:],
                            lhsT=w1t[:, t * D + 128 * m: t * D + 128 * m + 128],
                            rhs=x_t[:, h * GROUP:(h + 1) * GROUP],
                            start=(t == 0),
                            stop=(t == 3),
                        )
                    ht[m] = htpool.tile([128, GROUP], dt.bfloat16, tag="ht",
                                        name=f"ht_{j}_{m}")
                    # bias+relu+cast fused on the PSUM->SBUF move; split the
                    # two halves across ScalarE and VectorE
                    if m == 0:
                        nc.scalar.activation(ht[m][:], hp[:], FT.Relu,
                                             bias=b1t[:, m:m + 1])
                    else:
                        nc.vector.tensor_scalar(
                            ht[m][:], hp[:], b1t[:, m:m + 1], 0.0,
                            mybir.AluOpType.add, mybir.AluOpType.max)
                sp = sppool.tile([1, GROUP], dt.float32)
                nc.tensor.matmul(out=sp[:], lhsT=w2t[:, 0:1], rhs=ht[0][:],
                                 start=True, stop=False)
                nc.tensor.matmul(out=sp[:], lhsT=w2t[:, 1:2], rhs=ht[1][:],
                                 start=False, stop=True)
                sc_slice = sc[0:1, gg * GROUP:(gg + 1) * GROUP]
                if gg % 2 == 0:
                    nc.scalar.activation(sc_slice, sp[:], FT.Identity,
                                         bias=b2t[:, :])
                else:
                    nc.vector.tensor_scalar_add(sc_slice, sp[:], b2t[:, :])
            nc.sync.dma_start(out_d.ap()[0:1, jw0:jw0 + clen], sc[0:1, 0:clen])
            j0 += clen

    nc.compile()
    return nc


def _prep(inputs):
    z = np.asarray(inputs["z"], np.float32)
    ei = np.asarray(inputs["edge_index"]).astype(np.int64)
    w1 = np.asarray(inputs["w1"], np.float32)
    b1 = np.asarray(inputs["b1"], np.float32)
    w2 = np.asarray(inputs["w2"], np.float32)
    b2 = np.asarray(inputs["b2"], np.float32)

    zb = np.zeros((NPAD, D), BF16)
    zb[:N] = z.astype(BF16)

    w1t = np.ascontiguousarray(
        w1.T.reshape(4, 128, D).transpose(1, 0, 2).reshape(128, 4 * D)
    ).astype(BF16)
    b1t = np.ascontiguousarray(b1.reshape(2, 128).T)
    w2t = np.ascontiguousarray(w2.reshape(2, 128).T).astype(BF16)
    b2t = b2.reshape(1, 1).astype(np.float32)

    src = ei[0].reshape(NCORES, EPC)
    dst = ei[1].reshape(NCORES, EPC)
    bucket = (2 * (src >= H) + (dst >= H)).astype(np.int8)

    counts = np.stack([(bucket == b).sum(axis=1) for b in range(4)], axis=1)
    caps = [int(-(-counts[:, b].max() // GROUP)) * GROUP for b in range(4)]
    schedule = []
    for b in range(4):
        rem = caps[b]
        while rem > 0:
            c = min(CHUNK, rem)
            schedule.append((b, c))
            rem -= c
    L = sum(caps)
    G = L // GROUP
    Gb = G
    Sw = L

    isrc = np.zeros((NCORES, L), np.int16)
    idst = np.zeros((NCORES, L), np.int16)
    eids = np.full((NCORES, L), -1, np.int64)
    for c in range(NCORES):
        pos = 0
        for b in range(4):
            sel = np.nonzero(bucket[c] == b)[0]
            n = len(sel)
            isrc[c, pos:pos + n] = (src[c, sel] - H * (b >> 1)).astype(np.int16)
            idst[c, pos:pos + n] = (dst[c, sel] - H * (b & 1)).astype(np.int16)
            eids[c, pos:pos + n] = sel
            pos += caps[b]

    # wrapped layout [16, L/16] (idx j -> [j%16, j//16]), replicated to all
    # 128 partitions (8 gpsimd Q7 cores x 16 partitions each)
    isrc_w = np.ascontiguousarray(np.tile(
        isrc.reshape(NCORES, L // 16, 16).transpose(0, 2, 1), (1, 8, 1)))
    idst_w = np.ascontiguousarray(np.tile(
        idst.reshape(NCORES, L // 16, 16).transpose(0, 2, 1), (1, 8, 1)))

    in_maps = []
    for c in range(NCORES):
        in_maps.append({
            "z": zb, "isrc": isrc_w[c], "idst": idst_w[c],
            "w1t": w1t, "b1t": b1t, "w2t": w2t, "b2t": b2t,
        })
    meta = dict(schedule=tuple(schedule), L=L, Sw=Sw, Gb=Gb, eids=eids)
    return in_maps, meta


def _post(results, meta):
    L, Gb, eids = meta["L"], meta["Gb"], meta["eids"]
    out = np.empty((E, 1), np.float32)
    for c in range(NCORES):
        sj = results[c]["scores"].reshape(-1)[:L]
        valid = eids[c] >= 0
        res = np.empty(EPC, np.float32)
        res[eids[c, valid]] = sj[valid]
        out[c * EPC:(c + 1) * EPC, 0] = res
    return out


def kernel(**inputs) -> np.ndarray:
    from concourse import bass_utils

    in_maps, meta = _prep(inputs)
    key = (meta["schedule"], meta["L"], meta["Sw"])
    if key not in _cache:
        _cache[key] = _build(list(meta["schedule"]), meta["L"], meta["Sw"])
    nc = _cache[key]
    res = bass_utils.run_bass_kernel_spmd(nc, in_maps, core_ids=list(range(NCORES)))
    return _post(res.results, meta)


# revision 15
# speedup vs baseline: 2.7092x; 1.3713x over previous
"""MergeDecoder GNN edge-scorer for Trainium2 (8 NeuronCores).

Reference computation (per edge e):
    x  = [z[src[e]] ; z[dst[e]]]            # [512]
    h  = relu(w1 @ x + b1)                  # [256]
    out[e] = w2 @ h + b2                    # [1]

Strategy (matches sharding hint: data-parallel over E, replicate z/weights):
  - Host: cast z to bf16, shard edges 8-way, bucket-sort each shard's edges
    by (src>=32768, dst>=32768) so relative row indices fit the gather's
    int16 index format; build a unified static chunk schedule.
  - Device (per core): transposed dma_gather pulls z rows FEATURE-MAJOR into
    SBUF ([128 feat, 2, chunk_edges]); per 512-edge group, 8 accumulating
    matmuls compute h^T in PSUM; ScalarE fuses +b1, ReLU, bf16 cast on the
    PSUM->SBUF move; two M=1 matmuls against w2 produce scores [1,512] in
    PSUM; ScalarE adds b2 on the copy out. One DMA writes all scores back.
  - Host: de-interleave group-major scores, undo the bucket permutation.
"""

import os
import sys

for _p in ("/opt/trn_rl_repo",):
    if _p not in sys.path and os.path.isdir(_p):
        sys.path.insert(0, _p)

import numpy as np
import ml_dtypes

N, D, E = 50000, 256, 320000
NCORES = 8
EPC = E // NCORES          # 40000 edges per core
H = 32768                  # int16-safe split of node rows
NPAD = 50048               # z rows padded to a multiple of 128
CHUNK = 4096               # max edges per dma_gather call
GROUP = 512                # edges per matmul group (PSUM bank width in f32)

BF16 = ml_dtypes.bfloat16

_cache = {}


def _build(schedule, L, Sw, stages=3):
    """Trace the per-core Bass program for a given static chunk schedule."""
    from contextlib import ExitStack

    import concourse.bass as bass
    import concourse.mybir as mybir
    import concourse.tile as tile
    from concourse import bacc

    FT = mybir.ActivationFunctionType
    dt = mybir.dt

    nc = bacc.Bacc("TRN2", target_bir_lowering=False, debug=False,
                   enable_asserts=False)

    z_d = nc.dram_tensor("z", [NPAD, D], dt.bfloat16, kind="ExternalInput")
    isrc_d = nc.dram_tensor("isrc", [128, L // 16], dt.int16, kind="ExternalInput")
    idst_d = nc.dram_tensor("idst", [128, L // 16], dt.int16, kind="ExternalInput")
    w1t_d = nc.dram_tensor("w1t", [128, 4 * D], dt.bfloat16, kind="ExternalInput")
    b1t_d = nc.dram_tensor("b1t", [128, 2], dt.float32, kind="ExternalInput")
    w2t_d = nc.dram_tensor("w2t", [128, 2], dt.bfloat16, kind="ExternalInput")
    b2t_d = nc.dram_tensor("b2t", [1, 1], dt.float32, kind="ExternalInput")
    out_d = nc.dram_tensor("scores", [1, L], dt.float32, kind="ExternalOutput")

    with tile.TileContext(nc) as tc, ExitStack() as ctx:
        consts = ctx.enter_context(tc.tile_pool(name="consts", bufs=1))
        gpool_s = ctx.enter_context(tc.tile_pool(name="gath_s", bufs=8))
        gpool_d = ctx.enter_context(tc.tile_pool(name="gath_d", bufs=8))
        htpool = ctx.enter_context(tc.tile_pool(name="ht", bufs=4))
        hppool = ctx.enter_context(tc.tile_pool(name="hpsum", bufs=4, space="PSUM"))
        sppool = ctx.enter_context(tc.tile_pool(name="spsum", bufs=2, space="PSUM"))
        scpool = ctx.enter_context(tc.tile_pool(name="scores", bufs=2))

        w1t = consts.tile([128, 4 * D], dt.bfloat16)
        nc.sync.dma_start(w1t[:], w1t_d.ap())
        b1t = consts.tile([128, 2], dt.float32)
        nc.sync.dma_start(b1t[:], b1t_d.ap())
        w2t = consts.tile([128, 2], dt.bfloat16)
        nc.sync.dma_start(w2t[:], w2t_d.ap())
        b2t = consts.tile([1, 1], dt.float32)
        nc.sync.dma_start(b2t[:], b2t_d.ap())
        isrc = consts.tile([128, L // 16], dt.int16)
        nc.sync.dma_start(isrc[:], isrc_d.ap())
        idst = consts.tile([128, L // 16], dt.int16)
        nc.sync.dma_start(idst[:], idst_d.ap())

        z_lo = z_d.ap()[0:H, :]
        z_hi = z_d.ap()[H:NPAD, :]

        j0 = 0
        for (b, clen) in schedule:
            src_base = z_hi if (b >> 1) else z_lo
            dst_base = z_hi if (b & 1) else z_lo
            jw0 = j0 % L  # wraps only for repeated benchmark schedules

            if stages >= 3:
                sc = scpool.tile([1, CHUNK], dt.float32, tag="sc",
                                 name=f"sc_{j0}")
            for gg in range(clen // GROUP):
                j = jw0 + gg * GROUP
                # transpose-gather rx needs 2*n/16+2 of the 128
                # descriptor-ring slots per DMA engine -> n <= 896/call;
                # use one GROUP-sized call per matmul group
                xs = gpool_s.tile([128, 2 * GROUP], dt.bfloat16, tag="xs",
                                  name=f"xs_{j}")
                xd = gpool_d.tile([128, 2 * GROUP], dt.bfloat16, tag="xd",
                                  name=f"xd_{j}")
                for (x_t, base, idx) in ((xs, src_base, isrc),
                                         (xd, dst_base, idst)):
                    nc.gpsimd.dma_gather(
                        x_t[:].rearrange("p (h c) -> p h c", h=2),
                        base,
                        idx[:, j // 16:(j + GROUP) // 16],
                        GROUP,
                        GROUP,
                        D,
                        elem_step=D,
                        transpose=True,
                    )
                if stages < 2:
                    continue
                ht = [None, None]
                for m in range(2):
                    hp = hppool.tile([128, GROUP], dt.float32)
                    for t in range(4):
                        x_t = xs if t < 2 else xd
                        h = t & 1
                        nc.tensor.matmul(
                            out=hp[:],
                            lhsT=w1t[:, t * D + 128 * m: t * D + 128 * m + 128],
                            rhs=x_t[:, h * GROUP:(h + 1) * GROUP],
                            start=(t == 0),
                            stop=(t == 3),
                        )
                    ht[m] = htpool.tile([128, GROUP], dt.bfloat16, tag="ht",
                                        name=f"ht_{j}_{m}")
                    # bias+relu+cast fused on the PSUM->SBUF move; split the
                    # two halves across ScalarE and VectorE
                    if m == 0:
                        nc.scalar.activation(ht[m][:], hp[:], FT.Relu,
                                             bias=b1t[:, m:m + 1])
                    else:
                        nc.vector.tensor_scalar(
                            ht[m][:], hp[:], b1t[:, m:m + 1], 0.0,
                            mybir.AluOpType.add, mybir.AluOpType.max)
                if stages < 3:
                    continue
                sp = sppool.tile([1, GROUP], dt.float32)
                nc.tensor.matmul(out=sp[:], lhsT=w2t[:, 0:1], rhs=ht[0][:],
                                 start=True, stop=False)
                nc.tensor.matmul(out=sp[:], lhsT=w2t[:, 1:2], rhs=ht[1][:],
                                 start=False, stop=True)
                sc_slice = sc[0:1, gg * GROUP:(gg + 1) * GROUP]
                if gg % 2 == 0:
                    nc.scalar.activation(sc_slice, sp[:], FT.Identity,
                                         bias=b2t[:, :])
                else:
                    nc.vector.tensor_scalar_add(sc_slice, sp[:], b2t[:, :])
            if stages >= 3:
                nc.sync.dma_start(out_d.ap()[0:1, jw0:jw0 + clen],
                                  sc[0:1, 0:clen])
            j0 += clen
        if stages < 3:
            nc.sync.dma_start(out_d.ap()[0:1, 0:2], b1t[0:1, 0:2])

    nc.compile()
    return nc


def _prep(inputs):
    z = np.asarray(inputs["z"], np.float32)
    ei = np.asarray(inputs["edge_index"]).astype(np.int64)
    w1 = np.asarray(inputs["w1"], np.float32)
    b1 = np.asarray(inputs["b1"], np.float32)
    w2 = np.asarray(inputs["w2"], np.float32)
    b2 = np.asarray(inputs["b2"], np.float32)

    zb = np.zeros((NPAD, D), BF16)
    zb[:N] = z.astype(BF16)

    w1t = np.ascontiguousarray(
        w1.T.reshape(4, 128, D).transpose(1, 0, 2).reshape(128, 4 * D)
    ).astype(BF16)
    b1t = np.ascontiguousarray(b1.reshape(2, 128).T)
    w2t = np.ascontiguousarray(w2.reshape(2, 128).T).astype(BF16)
    b2t = b2.reshape(1, 1).astype(np.float32)

    src = ei[0].reshape(NCORES, EPC)
    dst = ei[1].reshape(NCORES, EPC)
    bucket = (2 * (src >= H) + (dst >= H)).astype(np.int8)

    counts = np.stack([(bucket == b).sum(axis=1) for b in range(4)], axis=1)
    caps = [int(-(-counts[:, b].max() // GROUP)) * GROUP for b in range(4)]
    schedule = []
    for b in range(4):
        rem = caps[b]
        while rem > 0:
            c = min(CHUNK, rem)
            schedule.append((b, c))
            rem -= c
    L = sum(caps)
    G = L // GROUP
    Gb = G
    Sw = L

    isrc = np.zeros((NCORES, L), np.int16)
    idst = np.zeros((NCORES, L), np.int16)
    eids = np.full((NCORES, L), -1, np.int64)
    for c in range(NCORES):
        pos = 0
        for b in range(4):
            sel = np.nonzero(bucket[c] == b)[0]
            n = len(sel)
            isrc[c, pos:pos + n] = (src[c, sel] - H * (b >> 1)).astype(np.int16)
            idst[c, pos:pos + n] = (dst[c, sel] - H * (b & 1)).astype(np.int16)
            eids[c, pos:pos + n] = sel
            pos += caps[b]

    # wrapped layout [16, L/16] (idx j -> [j%16, j//16]), replicated to all
    # 128 partitions (8 gpsimd Q7 cores x 16 partitions each)
    isrc_w = np.ascontiguousarray(np.tile(
        isrc.reshape(NCORES, L // 16, 16).transpose(0, 2, 1), (1, 8, 1)))
    idst_w = np.ascontiguousarray(np.tile(
        idst.reshape(NCORES, L // 16, 16).transpose(0, 2, 1), (1, 8, 1)))

    in_maps = []
    for c in range(NCORES):
        in_maps.append({
            "z": zb, "isrc": isrc_w[c], "idst": idst_w[c],
            "w1t": w1t, "b1t": b1t, "w2t": w2t, "b2t": b2t,
        })
    meta = dict(schedule=tuple(schedule), L=L, Sw=Sw, Gb=Gb, eids=eids)
    return in_maps, meta


def _post(results, meta):
    L, Gb, eids = meta["L"], meta["Gb"], meta["eids"]
    out = np.empty((E, 1), np.float32)
    for c in range(NCORES):
        sj = results[c]["scores"].reshape(-1)[:L]
        valid = eids[c] >= 0
        res = np.empty(EPC, np.float32)
        res[eids[c, valid]] = sj[valid]
        out[c * EPC:(c + 1) * EPC, 0] = res
    return out


def kernel(**inputs) -> np.ndarray:
    from concourse import bass_utils

    in_maps, meta = _prep(inputs)
    key = (meta["schedule"], meta["L"], meta["Sw"])
    if key not in _cache:
        _cache[key] = _build(list(meta["schedule"]), meta["L"], meta["Sw"])
    nc = _cache[key]
    res = bass_utils.run_bass_kernel_spmd(nc, in_maps, core_ids=list(range(NCORES)))
    return _post(res.results, meta)
